# revision 28
# baseline (speedup 1.0000x reference)
"""Trainium2 Bass kernel for LogicDense (soft differentiable logic layer).

Computation: out[n, j] = c0[j] + c1[j]*a + c2[j]*b + c3[j]*a*b
  where a = x[n, idx0[j]], b = x[n, idx1[j]] and
  coeff[j] = softmax(weight[j]) @ T  (T = 16x4 logic-op coefficient table).

Strategy (8 NeuronCores, sharded over out_dim):
  - Each core owns 1024 out-neurons; x is replicated as xT (in_dim, batch)
    in fp16 (precision-validated: max rel err ~6e-3 << 2e-2 gate).
  - Per 128-neuron chunk one SWDGE dma_gather pulls the 256 needed xT rows
    (a|b interleaved) as 8 KiB descriptors -> (128, 2, 4096) fp16 tile.
  - DVE computes m = a*b in fp16 (2x packed mode).
  - PE accumulates diag(c1)@a + diag(c2)@b + diag(c3)@m into PSUM f32
    (per-partition scaling via diagonal stationary matrices, fp16 full rate).
  - ScalarE folds "+ c0" into the PSUM->SBUF copy (Identity, per-partition
    bias) casting to fp16.
  - Stores are fully contiguous 1 MiB runs into outT (out_dim, batch);
    the host transposes/concats shards into the final (batch, out_dim).
  - softmax(weight)@[T|1] coefficients are computed on device (Exp on
    ScalarE, 8 tiny PE matmuls against (16,5) table, DVE reciprocal), and
    embedded into diagonal stationaries by scaling a 128x128 identity.
"""

import numpy as np

BATCH, IN_DIM, OUT_DIM = 4096, 4096, 8192
N_CORES = 8
OSH = OUT_DIM // N_CORES    # 1024 out-neurons per core
NCHUNK = OSH // 128         # 8 chunks of 128 neurons
NQ = 4                      # batch quarters
QB = BATCH // NQ            # 1024 batch cols per quarter

# difflogic bin_op_s coefficient table: op_i(a,b) = T[i,0] + T[i,1]*a +
# T[i,2]*b + T[i,3]*a*b
_T = np.array([
    [0.0,  0.0,  0.0,  0.0],
    [0.0,  0.0,  0.0,  1.0],
    [0.0,  1.0,  0.0, -1.0],
    [0.0,  1.0,  0.0,  0.0],
    [0.0,  0.0,  1.0, -1.0],
    [0.0,  0.0,  1.0,  0.0],
    [0.0,  1.0,  1.0, -2.0],
    [0.0,  1.0,  1.0, -1.0],
    [1.0, -1.0, -1.0,  1.0],
    [1.0, -1.0, -1.0,  2.0],
    [1.0,  0.0, -1.0,  0.0],
    [1.0,  0.0, -1.0,  1.0],
    [1.0, -1.0,  0.0,  0.0],
    [1.0, -1.0,  0.0,  1.0],
    [1.0,  0.0,  0.0, -1.0],
    [1.0,  0.0,  0.0,  0.0],
], dtype=np.float32)

_CACHE = {}


def build_program(repeat=None, variant="full"):
    """Build + compile the per-core Bass program (cached per process).

    repeat=K wraps the main gather/compute/store loop in a device-side
    For_i loop that runs it K times — used only for timing (the work is
    idempotent), never for the real kernel() path.

    variant: "full" (default, the real kernel) or an ablation for perf
    decomposition — "gatheronly", "nogather", "nostore", "empty".
    """
    key = ("nc", repeat, variant)
    if key in _CACHE:
        return _CACHE[key]

    import concourse.tile as tile
    import concourse.mybir as mybir
    from concourse import bacc

    dt = mybir.dt
    f32 = dt.float32
    f16 = dt.float16
    Alu = mybir.AluOpType
    Act = mybir.ActivationFunctionType

    nc = bacc.Bacc("TRN2", target_bir_lowering=False, debug=False,
                   num_devices=N_CORES, num_swdge_queues=4)
    xT = nc.dram_tensor("xT", [IN_DIM, BATCH], f16, kind="ExternalInput").ap()
    idxs = nc.dram_tensor("idxs", [128, NCHUNK * 16], dt.int16,
                          kind="ExternalInput").ap()
    wT = nc.dram_tensor("wT", [16, OSH], f32, kind="ExternalInput").ap()
    tmat = nc.dram_tensor("tmat", [16, 5], f32, kind="ExternalInput").ap()
    ident = nc.dram_tensor("ident", [128, 128], f16,
                           kind="ExternalInput").ap()
    out = nc.dram_tensor("out", [OSH, BATCH], f16, kind="ExternalOutput").ap()
    # neuron (= outT row) r is processed as chunk r % NCHUNK, partition
    # r // NCHUNK, so a 4-chunk store group writes 32 KiB contiguous per
    # partition (rows p*8+4g .. p*8+4g+3).
    out_r = out.rearrange("(p c) n -> p c n", c=NCHUNK)

    with tile.TileContext(nc) as tc:
        with (
            tc.tile_pool(name="const", bufs=1) as constp,
            tc.tile_pool(name="coef", bufs=1) as cpool,
            tc.tile_pool(name="gather", bufs=5) as gpool,
            tc.tile_pool(name="m", bufs=2) as mpool,
            tc.tile_pool(name="stage", bufs=3) as spool,
            tc.tile_pool(name="po", bufs=3, space="PSUM") as pspool,
            tc.tile_pool(name="pu", bufs=2, space="PSUM") as pupool,
        ):
            idx_sb = constp.tile([128, NCHUNK * 16], dt.int16)
            nc.sync.dma_start(idx_sb[:], idxs)
            tmat_sb = constp.tile([16, 5], f32)
            nc.sync.dma_start(tmat_sb[:], tmat)
            ident_sb = constp.tile([128, 128], f16)
            nc.sync.dma_start(ident_sb[:], ident)

            # --- coefficients: u = exp(w^T).T @ [T|1]; cnorm = u[:, :4]/u[:, 4]
            expw = cpool.tile([16, OSH], f32)
            nc.sync.dma_start(expw[:], wT)
            nc.scalar.activation(expw[:], expw[:], Act.Exp)
            u_all = cpool.tile([128, NCHUNK, 5], f32)
            for c in range(NCHUNK):
                pu = pupool.tile([128, 5], f32)
                nc.tensor.matmul(pu[:], expw[:, c * 128:(c + 1) * 128],
                                 tmat_sb[:], start=True, stop=True)
                nc.scalar.activation(u_all[:, c, :], pu[:], Act.Copy)
            rcp = cpool.tile([128, NCHUNK], f32)
            nc.vector.reciprocal(rcp[:], u_all[:, :, 4])
            cnorm = cpool.tile([128, NCHUNK, 4], f32)
            for k in range(4):
                nc.vector.tensor_tensor(cnorm[:, :, k], u_all[:, :, k],
                                        rcp[:], Alu.mult)
            # diagonal stationaries: dmat[:, c, :] = diag(c1 of chunk c)
            dmat = cpool.tile([128, NCHUNK, 128], f16)
            for c in range(NCHUNK):
                nc.vector.tensor_scalar(dmat[:, c, :], ident_sb[:],
                                        cnorm[:, c, 1:2], None, Alu.mult)

            # --- main loop over 8 chunks of 128 out-neurons
            gab_fixed = None
            if variant == "nogather":
                gab_fixed = constp.tile([128, 2, BATCH], f16)
                nc.vector.memset(gab_fixed[:], 0)

            # pure-store bandwidth experiments: store a constant staged tile
            # every iteration (no WAR, no compute) via different DMA paths
            stc = None
            if variant.startswith("st_") or variant == "nocompute":
                stc = constp.tile([128, NCHUNK, NQ, 2, 512], f16)
                nc.vector.memset(stc[:], 0)

            def st_loop():
                if variant == "st_hw1x8":
                    nc.sync.dma_start(out_r[:, :, :], stc[:])
                elif variant == "st_hw2x4":
                    for g in range(2):
                        nc.sync.dma_start(out_r[:, 4 * g:4 * (g + 1), :],
                                          stc[:, 4 * g:4 * (g + 1)])
                elif variant == "st_sw2x4":
                    for g in range(2):
                        nc.gpsimd.dma_start(out_r[:, 4 * g:4 * (g + 1), :],
                                            stc[:, 4 * g:4 * (g + 1)])
                elif variant == "st_mix2x4":
                    nc.sync.dma_start(out_r[:, 0:4, :], stc[:, 0:4])
                    nc.scalar.dma_start(out_r[:, 4:8, :], stc[:, 4:8])
                else:
                    raise ValueError(variant)

            def main_group(g):
                if variant == "nocompute":
                    for cc in range(2):
                        c = 2 * g + cc
                        gab = gpool.tile([128, 2, BATCH], f16, tag="gab")
                        nc.gpsimd.dma_gather(gab[:], xT,
                                             idx_sb[:, c * 16:(c + 1) * 16],
                                             256, 256, BATCH,
                                             queue_num=c % 4)
                    eng = nc.sync if g % 2 == 0 else nc.scalar
                    eng.dma_start(out_r[:, 2 * g:2 * (g + 1), :],
                                  stc[:, 2 * g:2 * (g + 1)])
                    return
                # one store group = 2 chunks staged together -> one 2 MiB
                # store whose per-partition runs are 16 KiB contiguous
                stage = spool.tile([128, 2, NQ, 2, 512], f16, tag="stage")
                if variant == "storeonly":
                    nc.vector.memset(stage[:], 0)
                    nc.sync.dma_start(out_r[:, 2 * g:2 * (g + 1), :], stage[:])
                    return
                for cc in range(2):
                    c = 2 * g + cc
                    if variant == "nogather":
                        gab = gab_fixed
                    else:
                        gab = gpool.tile([128, 2, BATCH], f16, tag="gab")
                        nc.gpsimd.dma_gather(gab[:], xT,
                                             idx_sb[:, c * 16:(c + 1) * 16],
                                             256, 256, BATCH,
                                             queue_num=c % 4)
                    if variant == "gatheronly":
                        continue
                    a = gab[:, 0, :]
                    b = gab[:, 1, :]
                    # out = c0 + c1*a + m2,  m2 = (c3*a + c2) * b
                    t2 = mpool.tile([128, BATCH], f16, tag="t2")
                    mt = mpool.tile([128, BATCH], f16, tag="mt")
                    for u in range(NQ):
                        sl = slice(u * QB, (u + 1) * QB)
                        nc.vector.tensor_scalar(t2[:, sl], a[:, sl],
                                                cnorm[:, c, 3:4],
                                                cnorm[:, c, 2:3],
                                                Alu.mult, Alu.add)
                        nc.vector.tensor_tensor(mt[:, sl], t2[:, sl], b[:, sl],
                                                Alu.mult)
                        po = pspool.tile([128, 2, 512], f32, tag="po")
                        for k, src in ((0, a), (1, mt[:])):
                            lhs = dmat[:, c, :] if k == 0 else ident_sb[:]
                            for s in range(2):
                                fs = slice(u * QB + s * 512,
                                           u * QB + (s + 1) * 512)
                                nc.tensor.matmul(po[:, s, :], lhs,
                                                 src[:, fs], start=(k == 0),
                                                 stop=(k == 1))
                        nc.scalar.activation(stage[:, cc, u], po[:],
                                             Act.Identity,
                                             bias=cnorm[:, c, 0:1], scale=1.0)
                if variant not in ("nostore", "gatheronly"):
                    # alternate the two HWDGE rings (SP / ACT) across groups
                    eng = nc.sync if g % 2 == 0 else nc.scalar
                    eng.dma_start(out_r[:, 2 * g:2 * (g + 1), :], stage[:])

            def main_loop():
                if variant == "empty":
                    return
                if variant.startswith("st_"):
                    st_loop()
                    return
                for g in range(NCHUNK // 2):
                    main_group(g)

            if repeat is None:
                main_loop()
            elif repeat < 0:        # python-unrolled (timeline-sim only)
                for _ in range(-repeat):
                    main_loop()
            else:
                with tc.For_i(0, repeat, 1):
                    main_loop()

    nc.compile()
    _CACHE[key] = nc
    return nc


_PERM = ((np.arange(OSH) % 128) * NCHUNK + np.arange(OSH) // 128)


def _wrap_idxs(i0, i1):
    """Per-chunk SWDGE int16 index table: chunk c, partition p covers
    neuron p*NCHUNK + c; call order is [a rows | b rows] (256 idxs).
    Idx j of a call lives at [p, c*16 + j//16] for p % 16 == j % 16,
    replicated across the eight 16-partition groups."""
    i0 = i0.reshape(128, NCHUNK)
    i1 = i1.reshape(128, NCHUNK)
    tab = np.empty((128, NCHUNK * 16), np.int16)
    for c in range(NCHUNK):
        blk = np.concatenate([i0[:, c], i1[:, c]]).astype(np.int16)
        w = blk.reshape(16, 16).T            # [p16, s16]
        tab[:, c * 16:(c + 1) * 16] = np.tile(w, (8, 1))
    return tab


def neuron_order(indices):
    """Global neuron processing order: sorted by the a-side row index so
    gather descriptors read ascending HBM addresses (row locality)."""
    return np.argsort(np.asarray(indices)[0], kind="stable")


def prepare_in_maps(x, indices, weight):
    x = np.asarray(x, np.float32)
    indices = np.asarray(indices)
    weight = np.asarray(weight, np.float32)
    order = neuron_order(indices)
    xT16 = np.ascontiguousarray(x.T).astype(np.float16)
    tm = np.ascontiguousarray(np.concatenate(
        [_T, np.ones((16, 1), np.float32)], axis=1))
    idm = np.eye(128, dtype=np.float16)
    in_maps = []
    for c in range(N_CORES):
        sel = order[c * OSH:(c + 1) * OSH]
        in_maps.append({
            "xT": xT16,
            "idxs": _wrap_idxs(indices[0][sel], indices[1][sel]),
            "wT": np.ascontiguousarray(weight[sel].T[:, _PERM]),
            "tmat": tm, "ident": idm,
        })
    return in_maps


def kernel(x, indices, weight):
    from concourse.bass_utils import run_bass_kernel_spmd

    nc = build_program()
    in_maps = prepare_in_maps(x, indices, weight)
    res = run_bass_kernel_spmd(nc, in_maps, core_ids=list(range(N_CORES)))
    order = neuron_order(indices)
    full = np.empty((BATCH, OUT_DIM), np.float32)
    for c in range(N_CORES):
        full[:, order[c * OSH:(c + 1) * OSH]] = np.asarray(
            res.results[c]["out"]).astype(np.float32).T
    return full


# revision 31
# speedup vs baseline: 1.0871x; 1.0871x over previous
"""Trainium2 Bass kernel for LogicDense (soft differentiable logic layer).

Computation: out[n, j] = c0[j] + c1[j]*a + c2[j]*b + c3[j]*a*b
  where a = x[n, idx0[j]], b = x[n, idx1[j]] and
  coeff[j] = softmax(weight[j]) @ T  (T = 16x4 logic-op coefficient table).

Strategy (8 NeuronCores, sharded over out_dim; DMA-floor-bound at ~84us):
  - Each core owns 1024 out-neurons; x is replicated as xT (in_dim, batch)
    in fp16 (precision-validated: max rel err ~6e-3 << 2e-2 gate).
  - Per 128-neuron chunk one SWDGE dma_gather pulls the 256 needed xT rows
    (a|b interleaved) as 8 KiB descriptors -> (128, 2, 4096) fp16 tile.
    Neurons are globally sorted by a-row (neuron_order) and mapped so that
    outT row p*8+c is processed as (chunk c, partition p).
  - Polynomial restructured as out = c0 + c1*a + (c3*a + c2)*b: DVE does
    the two fp16 passes (tensor_scalar 4x, tensor_tensor 2x); PE
    accumulates diag(c1)@a + I@m2 into PSUM f32 (fp16 full-rate matmuls,
    2 stationary loads per quarter); ScalarE folds "+ c0" into the
    PSUM->SBUF copy (Identity, per-partition bias) casting to fp16.
  - Stores write 2-chunk groups of outT (out_dim, batch) as 16 KiB
    contiguous per-partition runs, alternating the two HWDGE rings; the
    host transposes/permutes shards into the final (batch, out_dim).
  - softmax(weight)@[T|1] coefficients are computed on device (Exp on
    ScalarE, 8 tiny PE matmuls against (16,5) table, DVE reciprocal), and
    c1 embedded into diagonal stationaries by scaling a 128x128 identity.
  - Per-core HBM traffic is 16 MiB gathered reads + 8 MiB writes; measured
    device DMA streams run ~320-330 GB/s, so ~84us/iter is the roofline.
"""

import numpy as np

BATCH, IN_DIM, OUT_DIM = 4096, 4096, 8192
N_CORES = 8
OSH = OUT_DIM // N_CORES    # 1024 out-neurons per core
NCHUNK = OSH // 128         # 8 chunks of 128 neurons
NQ = 4                      # batch quarters
QB = BATCH // NQ            # 1024 batch cols per quarter

# difflogic bin_op_s coefficient table: op_i(a,b) = T[i,0] + T[i,1]*a +
# T[i,2]*b + T[i,3]*a*b
_T = np.array([
    [0.0,  0.0,  0.0,  0.0],
    [0.0,  0.0,  0.0,  1.0],
    [0.0,  1.0,  0.0, -1.0],
    [0.0,  1.0,  0.0,  0.0],
    [0.0,  0.0,  1.0, -1.0],
    [0.0,  0.0,  1.0,  0.0],
    [0.0,  1.0,  1.0, -2.0],
    [0.0,  1.0,  1.0, -1.0],
    [1.0, -1.0, -1.0,  1.0],
    [1.0, -1.0, -1.0,  2.0],
    [1.0,  0.0, -1.0,  0.0],
    [1.0,  0.0, -1.0,  1.0],
    [1.0, -1.0,  0.0,  0.0],
    [1.0, -1.0,  0.0,  1.0],
    [1.0,  0.0,  0.0, -1.0],
    [1.0,  0.0,  0.0,  0.0],
], dtype=np.float32)

_CACHE = {}


def build_program(repeat=None, variant="full", unroll=1):
    """Build + compile the per-core Bass program (cached per process).

    repeat=K wraps the main gather/compute/store loop in a device-side
    For_i loop that runs it K times — used only for timing (the work is
    idempotent), never for the real kernel() path. unroll=U replicates
    the body U times inside each For_i iteration (K*U body runs total),
    amortizing any loop-boundary synchronization.

    variant: "full" (default, the real kernel) or an ablation for perf
    decomposition — "gatheronly", "nogather", "nostore", "empty".
    """
    key = ("nc", repeat, variant, unroll)
    if key in _CACHE:
        return _CACHE[key]

    import concourse.tile as tile
    import concourse.mybir as mybir
    from concourse import bacc

    dt = mybir.dt
    f32 = dt.float32
    f16 = dt.float16
    Alu = mybir.AluOpType
    Act = mybir.ActivationFunctionType

    nc = bacc.Bacc("TRN2", target_bir_lowering=False, debug=False,
                   num_devices=N_CORES, num_swdge_queues=4)
    xT = nc.dram_tensor("xT", [IN_DIM, BATCH], f16, kind="ExternalInput").ap()
    idxs = nc.dram_tensor("idxs", [128, NCHUNK * 16], dt.int16,
                          kind="ExternalInput").ap()
    wT = nc.dram_tensor("wT", [16, OSH], f32, kind="ExternalInput").ap()
    tmat = nc.dram_tensor("tmat", [16, 5], f32, kind="ExternalInput").ap()
    ident = nc.dram_tensor("ident", [128, 128], f16,
                           kind="ExternalInput").ap()
    out = nc.dram_tensor("out", [OSH, BATCH], f16, kind="ExternalOutput").ap()
    # neuron (= outT row) r is processed as chunk r % NCHUNK, partition
    # r // NCHUNK, so a 4-chunk store group writes 32 KiB contiguous per
    # partition (rows p*8+4g .. p*8+4g+3).
    out_r = out.rearrange("(p c) n -> p c n", c=NCHUNK)

    with tile.TileContext(nc) as tc:
        with (
            tc.tile_pool(name="const", bufs=1) as constp,
            tc.tile_pool(name="coef", bufs=1) as cpool,
            tc.tile_pool(name="gather", bufs=5) as gpool,
            tc.tile_pool(name="m", bufs=2) as mpool,
            tc.tile_pool(name="stage", bufs=3) as spool,
            tc.tile_pool(name="po", bufs=3, space="PSUM") as pspool,
            tc.tile_pool(name="pu", bufs=2, space="PSUM") as pupool,
        ):
            idx_sb = constp.tile([128, NCHUNK * 16], dt.int16)
            nc.sync.dma_start(idx_sb[:], idxs)
            tmat_sb = constp.tile([16, 5], f32)
            nc.sync.dma_start(tmat_sb[:], tmat)
            ident_sb = constp.tile([128, 128], f16)
            nc.sync.dma_start(ident_sb[:], ident)

            # --- coefficients: u = exp(w^T).T @ [T|1]; cnorm = u[:, :4]/u[:, 4]
            expw = cpool.tile([16, OSH], f32)
            nc.sync.dma_start(expw[:], wT)
            nc.scalar.activation(expw[:], expw[:], Act.Exp)
            u_all = cpool.tile([128, NCHUNK, 5], f32)
            for c in range(NCHUNK):
                pu = pupool.tile([128, 5], f32)
                nc.tensor.matmul(pu[:], expw[:, c * 128:(c + 1) * 128],
                                 tmat_sb[:], start=True, stop=True)
                nc.scalar.activation(u_all[:, c, :], pu[:], Act.Copy)
            rcp = cpool.tile([128, NCHUNK], f32)
            nc.vector.reciprocal(rcp[:], u_all[:, :, 4])
            cnorm = cpool.tile([128, NCHUNK, 4], f32)
            for k in range(4):
                nc.vector.tensor_tensor(cnorm[:, :, k], u_all[:, :, k],
                                        rcp[:], Alu.mult)
            # diagonal stationaries: dmat[:, c, :] = diag(c1 of chunk c)
            dmat = cpool.tile([128, NCHUNK, 128], f16)
            for c in range(NCHUNK):
                nc.vector.tensor_scalar(dmat[:, c, :], ident_sb[:],
                                        cnorm[:, c, 1:2], None, Alu.mult)

            # --- main loop over 8 chunks of 128 out-neurons
            gab_fixed = None
            if variant == "nogather":
                gab_fixed = constp.tile([128, 2, BATCH], f16)
                nc.vector.memset(gab_fixed[:], 0)

            # pure-store bandwidth experiments: store a constant staged tile
            # every iteration (no WAR, no compute) via different DMA paths
            stc = None
            if variant.startswith("st_") or variant == "nocompute":
                stc = constp.tile([128, NCHUNK, NQ, 2, 512], f16)
                nc.vector.memset(stc[:], 0)

            def st_loop():
                if variant == "st_hw1x8":
                    nc.sync.dma_start(out_r[:, :, :], stc[:])
                elif variant == "st_hw2x4":
                    for g in range(2):
                        nc.sync.dma_start(out_r[:, 4 * g:4 * (g + 1), :],
                                          stc[:, 4 * g:4 * (g + 1)])
                elif variant == "st_sw2x4":
                    for g in range(2):
                        nc.gpsimd.dma_start(out_r[:, 4 * g:4 * (g + 1), :],
                                            stc[:, 4 * g:4 * (g + 1)])
                elif variant == "st_mix2x4":
                    nc.sync.dma_start(out_r[:, 0:4, :], stc[:, 0:4])
                    nc.scalar.dma_start(out_r[:, 4:8, :], stc[:, 4:8])
                else:
                    raise ValueError(variant)

            def main_group(g):
                if variant == "nocompute":
                    for cc in range(2):
                        c = 2 * g + cc
                        gab = gpool.tile([128, 2, BATCH], f16, tag="gab")
                        nc.gpsimd.dma_gather(gab[:], xT,
                                             idx_sb[:, c * 16:(c + 1) * 16],
                                             256, 256, BATCH,
                                             queue_num=c % 4)
                    eng = nc.sync if g % 2 == 0 else nc.scalar
                    eng.dma_start(out_r[:, 2 * g:2 * (g + 1), :],
                                  stc[:, 2 * g:2 * (g + 1)])
                    return
                # one store group = 2 chunks staged together -> one 2 MiB
                # store whose per-partition runs are 16 KiB contiguous
                stage = spool.tile([128, 2, NQ, 2, 512], f16, tag="stage")
                if variant == "storeonly":
                    nc.vector.memset(stage[:], 0)
                    nc.sync.dma_start(out_r[:, 2 * g:2 * (g + 1), :], stage[:])
                    return
                for cc in range(2):
                    c = 2 * g + cc
                    if variant == "nogather":
                        gab = gab_fixed
                    else:
                        gab = gpool.tile([128, 2, BATCH], f16, tag="gab")
                        nc.gpsimd.dma_gather(gab[:], xT,
                                             idx_sb[:, c * 16:(c + 1) * 16],
                                             256, 256, BATCH,
                                             queue_num=c % 4)
                    if variant == "gatheronly":
                        continue
                    a = gab[:, 0, :]
                    b = gab[:, 1, :]
                    # out = c0 + c1*a + m2,  m2 = (c3*a + c2) * b
                    t2 = mpool.tile([128, BATCH], f16, tag="t2")
                    mt = mpool.tile([128, BATCH], f16, tag="mt")
                    for u in range(NQ):
                        sl = slice(u * QB, (u + 1) * QB)
                        nc.vector.tensor_scalar(t2[:, sl], a[:, sl],
                                                cnorm[:, c, 3:4],
                                                cnorm[:, c, 2:3],
                                                Alu.mult, Alu.add)
                        nc.vector.tensor_tensor(mt[:, sl], t2[:, sl], b[:, sl],
                                                Alu.mult)
                        po = pspool.tile([128, 2, 512], f32, tag="po")
                        for k, src in ((0, a), (1, mt[:])):
                            lhs = dmat[:, c, :] if k == 0 else ident_sb[:]
                            for s in range(2):
                                fs = slice(u * QB + s * 512,
                                           u * QB + (s + 1) * 512)
                                nc.tensor.matmul(po[:, s, :], lhs,
                                                 src[:, fs], start=(k == 0),
                                                 stop=(k == 1))
                        nc.scalar.activation(stage[:, cc, u], po[:],
                                             Act.Identity,
                                             bias=cnorm[:, c, 0:1], scale=1.0)
                if variant not in ("nostore", "gatheronly"):
                    # alternate the two HWDGE rings (SP / ACT) across groups
                    eng = nc.sync if g % 2 == 0 else nc.scalar
                    eng.dma_start(out_r[:, 2 * g:2 * (g + 1), :], stage[:])

            def main_loop():
                if variant == "empty":
                    return
                if variant.startswith("st_"):
                    st_loop()
                    return
                for g in range(NCHUNK // 2):
                    main_group(g)

            if repeat is None:
                main_loop()
            elif repeat < 0:        # python-unrolled (timeline-sim only)
                for _ in range(-repeat):
                    main_loop()
            else:
                with tc.For_i(0, repeat, 1):
                    for _ in range(unroll):
                        main_loop()

    nc.compile()
    _CACHE[key] = nc
    return nc


_PERM = ((np.arange(OSH) % 128) * NCHUNK + np.arange(OSH) // 128)


def _wrap_idxs(i0, i1):
    """Per-chunk SWDGE int16 index table: chunk c, partition p covers
    neuron p*NCHUNK + c; call order is [a rows | b rows] (256 idxs).
    Idx j of a call lives at [p, c*16 + j//16] for p % 16 == j % 16,
    replicated across the eight 16-partition groups."""
    i0 = i0.reshape(128, NCHUNK)
    i1 = i1.reshape(128, NCHUNK)
    tab = np.empty((128, NCHUNK * 16), np.int16)
    for c in range(NCHUNK):
        blk = np.concatenate([i0[:, c], i1[:, c]]).astype(np.int16)
        w = blk.reshape(16, 16).T            # [p16, s16]
        tab[:, c * 16:(c + 1) * 16] = np.tile(w, (8, 1))
    return tab


def neuron_order(indices):
    """Global neuron processing order: sorted by the a-side row index so
    gather descriptors read ascending HBM addresses (row locality)."""
    return np.argsort(np.asarray(indices)[0], kind="stable")


def prepare_in_maps(x, indices, weight):
    x = np.asarray(x, np.float32)
    indices = np.asarray(indices)
    weight = np.asarray(weight, np.float32)
    order = neuron_order(indices)
    xT16 = np.ascontiguousarray(x.T).astype(np.float16)
    tm = np.ascontiguousarray(np.concatenate(
        [_T, np.ones((16, 1), np.float32)], axis=1))
    idm = np.eye(128, dtype=np.float16)
    in_maps = []
    for c in range(N_CORES):
        sel = order[c * OSH:(c + 1) * OSH]
        in_maps.append({
            "xT": xT16,
            "idxs": _wrap_idxs(indices[0][sel], indices[1][sel]),
            "wT": np.ascontiguousarray(weight[sel].T[:, _PERM]),
            "tmat": tm, "ident": idm,
        })
    return in_maps


def kernel(x, indices, weight):
    from concourse.bass_utils import run_bass_kernel_spmd

    nc = build_program()
    in_maps = prepare_in_maps(x, indices, weight)
    res = run_bass_kernel_spmd(nc, in_maps, core_ids=list(range(N_CORES)))
    order = neuron_order(indices)
    full = np.empty((BATCH, OUT_DIM), np.float32)
    for c in range(N_CORES):
        full[:, order[c * OSH:(c + 1) * OSH]] = np.asarray(
            res.results[c]["out"]).astype(np.float32).T
    return full
